# revision 1
# baseline (speedup 1.0000x reference)
"""CenterLoss kernel for Trainium2, 8 NeuronCores, data-parallel over the batch.

Reference computation (B=65536, D=512, C=1024):
    sums_c  = segment_sum(x, t)                 # [C, D]
    counts  = bincount(t)                       # [C]
    centers = sums / max(counts, 1)
    loss    = 0.5 * sum_i ||x_i - centers[t_i]||^2

Algebraic rewrite (exact, incl. empty classes):
    loss = 0.5 * ( sum_i ||x_i||^2  -  sum_c ||S_c||^2 / max(n_c, 1) )

Per core (8192 samples), per 256-sample supertile u:
  - fp8 one-hot of the 2x128 targets, r-major layout [p, r, C] (two
    contiguous [128, 1024] DVE writes);
  - 8 DoubleRow fp8 matmuls (K=256) accumulate segment sums for all 1024
    classes across all 8 PSUM banks;
  - sum(x^2) via one ACT Square (accum_out) per 1024-sample DMA group;
  - fp8 cast of x split across engines to keep every engine under the
    DMA pace.
One-hots stay resident (64 KB/partition); the epilogue reduces them to
exact per-class counts with 64 FD-512 ones-matmuls.  Cross-core:
ReduceScatter of [C, D+1] (S || counts), per-core class-shard partial of
the second term, then a tiny ReduceScatter of the replicated [8, 2]
scalar partials (cheaper than AllReduce).
"""

import numpy as np

from concourse import bass, bacc, tile, mybir, bass_utils

B, D, C = 65536, 512, 1024
N_CORES = 8
BL = B // N_CORES          # samples per core
P = 128                    # partitions / tile rows
NT = BL // P               # 64 sample tiles per core
G = 8                      # tiles per DMA group
NG = NT // G               # 8 groups
NU = NT // 2               # 32 supertiles
NCHUNK = C // P            # 8 class chunks

_f32 = mybir.dt.float32
_bf16 = mybir.dt.bfloat16
_f16 = mybir.dt.float16
_i32 = mybir.dt.int32
_f8 = mybir.dt.float8e4
_f8e5 = mybir.dt.float8e5

_compiled = None


def _build(repeat=1):
    nc = bacc.Bacc("TRN2", target_bir_lowering=False, debug=False,
                   num_devices=N_CORES)

    x_d = nc.dram_tensor("x", [BL, D], _f32, kind="ExternalInput")
    t_d = nc.dram_tensor("t", [BL], _i32, kind="ExternalInput")
    iota_d = nc.dram_tensor("iota", [P, C], _f16, kind="ExternalInput")
    out_d = nc.dram_tensor("out", [1, 1], _f32, kind="ExternalOutput")

    rg = [list(range(N_CORES))]

    with tile.TileContext(nc) as tc:
        with (
            tc.tile_pool(name="const", bufs=1) as cpool,
            tc.tile_pool(name="xg", bufs=4) as xgpool,
            tc.tile_pool(name="work", bufs=6) as wpool,
            tc.tile_pool(name="psum", bufs=1, space="PSUM") as ppool,
            tc.tile_pool(name="dram", bufs=1, space="DRAM") as dpool,
        ):
            # ---- constants / persistent state ----
            iota_sb = cpool.tile([P, C], _f16, tag="iota")
            nc.sync.dma_start(iota_sb[:], iota_d.ap())

            ones_f32 = cpool.tile([P, 1], _f32, tag="ones_f32")
            nc.vector.memset(ones_f32[:], 1.0)
            # [p, 2, 16] DoubleRow ones-weight (r stride 16B satisfies the
            # step%16 constraint on weight APs)
            ones_f8 = cpool.tile([P, 32], _f8, tag="ones_f8")
            nc.vector.memset(ones_f8[:], 1.0)

            # sample (g, p, j): row = g*(P*G) + p*G + j  (8 KiB/partition
            # contiguous DMA lines); tile index k = g*G + j.
            t_i32 = cpool.tile([P, NT], _i32, tag="t_i32")
            nc.sync.dma_start(
                t_i32[:].rearrange("p (g j) -> p g j", j=G),
                t_d.ap().rearrange("(g p j) -> p g j", p=P, j=G))
            t_f32 = cpool.tile([P, NT], _f32, tag="t_f32")
            nc.vector.tensor_copy(t_f32[:], t_i32[:])

            # all supertile one-hots, r-major: [p, u, r, C] fp8 (exact 0/1)
            o_all = cpool.tile([P, NU * 2 * C], _f8, tag="o_all")

            # running sum-of-squares accumulator [P, 1] f32
            sq_acc = cpool.tile([P, 1], _f32, tag="sq_acc")
            nc.vector.memset(sq_acc[:], 0.0)

            # ---- PSUM: 8 banks accumulate segment sums over all tiles ----
            psum_s = [ppool.tile([P, D], _f32, tag=f"s{c}", name=f"psum_s{c}")
                      for c in range(NCHUNK)]

            xga = x_d.ap().rearrange("(g p j) d -> g p j d", p=P, j=G)

            # ---- PE warm-up burst ----
            # HAM keeps the PE at 1.2 GHz until ~3 us of *continuous*
            # execution; a DMA-paced loop never accumulates that, so pay a
            # one-time ~7 us burst of zero-weight matmuls (adds 0 to PSUM)
            # to reach 2.4 GHz.  Once warm, the loop's sub-us idle gaps stay
            # below the ~3.4 us re-throttle window.
            zero_w = cpool.tile([P, 32], _f8, tag="zero_w")
            nc.vector.memset(zero_w[:], 0.0)
            warm_x = cpool.tile([P, 2 * D], _f8, tag="warm_x")
            nc.vector.memset(warm_x[:], 0.0)
            z3 = zero_w[:].rearrange("p (r m) -> p r m", r=2)
            wx3 = warm_x[:].rearrange("p (r d) -> p r d", r=2)
            for w in range(32):
                nc.tensor.matmul(
                    psum_s[w % NCHUNK][0:16, :], lhsT=z3, rhs=wx3,
                    perf_mode=mybir.MatmulPerfMode.DoubleRow,
                    start=(w < NCHUNK), stop=(w >= 32 - NCHUNK),
                    skip_group_check=True,
                )

            # ---- main loop ----
            def main_loop():
                for g in range(NG):
                    xg = xgpool.tile([P, G * D], _f32, tag="xg")
                    nc.sync.dma_start(
                        xg[:].rearrange("p (j d) -> p j d", j=G), xga[g])

                    # one ACT Square + accum for the whole 4096-elem group
                    sqs = wpool.tile([P, G * D], _f8e5, tag="sqs")
                    sqp = wpool.tile([P, 1], _f32, tag="sqp")
                    nc.scalar.activation(
                        sqs[:], xg[:], mybir.ActivationFunctionType.Square,
                        accum_out=sqp[:, 0:1])
                    nc.vector.tensor_tensor(
                        sq_acc[:], sq_acc[:], sqp[:], mybir.AluOpType.add)

                    for h in range(G // 2):
                        u = g * (G // 2) + h
                        xv2 = xg[:, h * 2 * D:(h + 1) * 2 * D]   # [P, 1024]

                        # fp8 cast of the supertile pair; alternate DVE/ACT
                        # (measured: DVE 379ns, ACT 750ns per [128,1024];
                        # GPSIMD is far too slow for this)
                        xf8 = wpool.tile([P, 2 * D], _f8, tag="xf8")
                        if u % 2 == 0:
                            nc.vector.tensor_copy(xf8[:], xv2)
                        else:
                            nc.scalar.copy(xf8[:], xv2)

                        # fp8 one-hots, r-major contiguous [p, r, C]
                        ob = o_all[:, u * 2 * C:(u + 1) * 2 * C]
                        for r in range(2):
                            nc.vector.tensor_scalar(
                                ob[:, r * C:(r + 1) * C], iota_sb[:],
                                t_f32[:, 2 * u + r:2 * u + r + 1], None,
                                mybir.AluOpType.is_equal,
                            )

                        # segment-sum DoubleRow matmuls (K=256 per supertile)
                        x3 = xf8[:].rearrange("p (r d) -> p r d", r=2)
                        o3 = ob.rearrange("p (r c) -> p r c", r=2)
                        for c in range(NCHUNK):
                            nc.tensor.matmul(
                                psum_s[c][:],
                                lhsT=o3[:, :, c * P:(c + 1) * P],
                                rhs=x3,
                                perf_mode=mybir.MatmulPerfMode.DoubleRow,
                                start=(u == 0), stop=(u == NU - 1),
                            )

            if repeat == 1:
                main_loop()
            else:
                with tc.For_i(0, repeat, 1):
                    main_loop()

            # ---- epilogue: flush S, counts, sumsq ----
            # bf16 cross-core payload; counts stay exact (integers < 256).
            s_sb = cpool.tile([P, NCHUNK * D], _bf16, tag="s_sb")
            # flush banks 6,7 first (DVE) so PE count matmuls can start on
            # the freed banks while ACT flushes the rest.
            nc.vector.tensor_copy(s_sb[:, 6 * D:7 * D], psum_s[6][:])
            nc.vector.tensor_copy(s_sb[:, 7 * D:8 * D], psum_s[7][:])
            for c in range(6):
                if c % 3 == 2:
                    nc.vector.tensor_copy(s_sb[:, c * D:(c + 1) * D],
                                          psum_s[c][:])
                else:
                    nc.scalar.copy(s_sb[:, c * D:(c + 1) * D], psum_s[c][:])

            # counts: 64 FD-512 ones-matmuls over the resident one-hots
            ones3 = ones_f8[:].rearrange("p (r m) -> p r m", r=2)
            cnt_ps = [ppool.tile([16, D], _f32, tag=f"s{6 + jh}",
                                 name=f"cnt_ps{jh}") for jh in range(2)]
            o4 = o_all[:].rearrange("p (u r c) -> p u r c", u=NU, r=2)
            for u in range(NU):
                for jh in range(2):
                    nc.tensor.matmul(
                        cnt_ps[jh][:],
                        lhsT=ones3,
                        rhs=o4[:, u, :, jh * D:(jh + 1) * D],
                        perf_mode=mybir.MatmulPerfMode.DoubleRow,
                        start=(u == 0), stop=(u == NU - 1),
                    )
            cnt_sb = cpool.tile([1, C], _bf16, tag="cnt_sb")
            for jh in range(2):
                nc.vector.tensor_copy(cnt_sb[:, jh * D:(jh + 1) * D],
                                      cnt_ps[jh][0:1, :])

            # sumsq partial: reduce [P, 1] across partitions
            sq_ps = ppool.tile([1, 1], _f32, tag="s0", name="sq_ps")
            nc.tensor.matmul(sq_ps[:], lhsT=ones_f32[:], rhs=sq_acc[:, 0:1],
                             start=True, stop=True)

            # ---- assemble ReduceScatter input [C, D+1] = [S | counts] ----
            rs_in = dpool.tile([C, D + 1], _bf16, tag="rs_in")
            for c in range(NCHUNK):
                nc.sync.dma_start(rs_in[c * P:(c + 1) * P, 0:D],
                                  s_sb[:, c * D:(c + 1) * D])
            nc.sync.dma_start(rs_in[0:C, D:D + 1], cnt_sb[0:1, 0:C])

            rs_out = dpool.tile([C // N_CORES, D + 1], _bf16, tag="rs_out")
            nc.gpsimd.collective_compute(
                "ReduceScatter", mybir.AluOpType.add, replica_groups=rg,
                ins=[rs_in.opt()], outs=[rs_out.opt()],
            )

            # ---- per-core class-shard term: sum_c ||S_c||^2 / max(n_c,1) ----
            sh = cpool.tile([P, D + 1], _bf16, tag="sh")
            nc.sync.dma_start(sh[:], rs_out[:])

            q = cpool.tile([P, 1], _f32, tag="q")
            qscr = wpool.tile([P, D], _f32, tag="qscr")
            nc.vector.tensor_tensor(qscr[:], sh[:, 0:D], sh[:, 0:D],
                                    mybir.AluOpType.mult)
            nc.vector.tensor_reduce(q[:, 0:1], qscr[:],
                                    axis=mybir.AxisListType.X,
                                    op=mybir.AluOpType.add)
            nmax = cpool.tile([P, 1], _f32, tag="nmax")
            nc.vector.tensor_scalar_max(nmax[:], sh[:, D:D + 1], 1.0)
            rinv = cpool.tile([P, 1], _f32, tag="rinv")
            nc.vector.reciprocal(rinv[:], nmax[:])
            bpart = cpool.tile([P, 1], _f32, tag="bpart")
            nc.vector.tensor_tensor(bpart[:], q[:], rinv[:],
                                    mybir.AluOpType.mult)
            b_ps = ppool.tile([1, 1], _f32, tag="s1", name="b_ps")
            nc.tensor.matmul(b_ps[:], lhsT=ones_f32[:], rhs=bpart[:, 0:1],
                             start=True, stop=True)

            # ---- scalar cross-core reduce: RS of replicated [8, 2] ----
            par_sb = cpool.tile([1, 2 * N_CORES], _f32, tag="par_sb")
            for m in range(N_CORES):
                nc.vector.tensor_copy(par_sb[0:1, 2 * m:2 * m + 1], sq_ps[:])
                nc.vector.tensor_copy(par_sb[0:1, 2 * m + 1:2 * m + 2],
                                      b_ps[:])
            rs2_in = dpool.tile([N_CORES, 2], _f32, tag="rs2_in")
            nc.sync.dma_start(rs2_in[:], par_sb[:])
            rs2_out = dpool.tile([1, 2], _f32, tag="rs2_out")
            nc.gpsimd.collective_compute(
                "ReduceScatter", mybir.AluOpType.add, replica_groups=rg,
                ins=[rs2_in.opt()], outs=[rs2_out.opt()],
            )
            fin = cpool.tile([1, 2], _f32, tag="fin")
            nc.sync.dma_start(fin[:], rs2_out[:])

            loss_sb = cpool.tile([1, 1], _f32, tag="loss_sb")
            nc.vector.tensor_tensor(loss_sb[:], fin[0:1, 0:1], fin[0:1, 1:2],
                                    mybir.AluOpType.subtract)
            nc.vector.tensor_scalar_mul(loss_sb[:], loss_sb[:], 0.5)
            nc.sync.dma_start(out_d.ap(), loss_sb[:])

    nc.compile()
    return nc


def _get_compiled():
    global _compiled
    if _compiled is None:
        _compiled = _build()
    return _compiled


_IOTA = np.tile(np.arange(C, dtype=np.float16), (P, 1))


def make_in_maps(inputs, targets):
    x = np.ascontiguousarray(np.asarray(inputs, dtype=np.float32))
    t = np.ascontiguousarray(np.asarray(targets).astype(np.int32))
    assert x.shape == (B, D) and t.shape == (B,)
    return [
        {
            "x": x[c * BL:(c + 1) * BL],
            "t": t[c * BL:(c + 1) * BL],
            "iota": _IOTA,
        }
        for c in range(N_CORES)
    ]


def kernel(inputs, targets, num_classes=C, **_ignored):
    assert int(num_classes) == C
    nc = _get_compiled()
    res = bass_utils.run_bass_kernel_spmd(
        nc, make_in_maps(inputs, targets), core_ids=list(range(N_CORES)))
    return np.asarray(res.results[0]["out"], dtype=np.float32).reshape(())



# revision 2
# speedup vs baseline: 1.0299x; 1.0299x over previous
"""CenterLoss kernel for Trainium2, 8 NeuronCores, CLASS-sharded.

Reference computation (B=65536, D=512, C=1024):
    sums_c  = segment_sum(x, t)                 # [C, D]
    counts  = bincount(t)                       # [C]
    centers = sums / max(counts, 1)
    loss    = 0.5 * sum_i ||x_i - centers[t_i]||^2

Algebraic rewrite (exact, incl. empty classes):
    loss = 0.5 * ( sum_i ||x_i||^2  -  sum_c ||S_c||^2 / max(n_c, 1) )

Sharding: core m owns classes [128m, 128(m+1)).  kernel() routes each
sample to the core owning its class (host-side permutation — a sharding
choice; every byte of x still streams through the device exactly once).
Rows are padded to BLP=8448 per core with target=-1 (matches no class,
x=0 adds nothing to sum||x||^2).

Per core, per 768-sample group g (6 tiles of 128, 1.5 MiB DMA):
  - sum(x^2) via one ACT Square (accum_out, f32 exact);
  - per 256-sample supertile: fp8 cast of x (DVE),
    local 128-class one-hot (DVE is_equal), and ONE DoubleRow fp8
    matmul accumulating S[128, 512] in a single PSUM bank.
Cross-core work is only the final scalar: each core's
(sum_sq_partial, sum_c ||S_c||^2/n_c) pair, combined with one tiny
ReduceScatter.  No [C, D] collective exists in this sharding.

PE load is ~33 matmuls/iter (~3% busy) vs 256 in the batch-sharded
variant — the kernel is purely HBM-DMA-bound.
"""

import numpy as np

from concourse import bass, bacc, tile, mybir, bass_utils

B, D, C = 65536, 512, 1024
N_CORES = 8
CPL = C // N_CORES         # classes per core (= 128)
P = 128                    # partitions / tile rows
BLP = 8448                 # padded samples per core (max seed-0 count 8374)
NT = BLP // P              # 68 sample tiles per core
G = 6                      # tiles per DMA group (1.5 MiB)
NG = NT // G               # 11 groups
NU = NT // 2               # 33 supertiles

_f32 = mybir.dt.float32
_f16 = mybir.dt.float16
_i32 = mybir.dt.int32
_f8 = mybir.dt.float8e4
_f8e5 = mybir.dt.float8e5

_compiled = None


def _build(repeat=1):
    nc = bacc.Bacc("TRN2", target_bir_lowering=False, debug=False,
                   num_devices=N_CORES)

    x_d = nc.dram_tensor("x", [BLP, D], _f32, kind="ExternalInput")
    t_d = nc.dram_tensor("t", [BLP], _i32, kind="ExternalInput")
    iota_d = nc.dram_tensor("iota", [P, CPL], _f16, kind="ExternalInput")
    out_d = nc.dram_tensor("out", [1, 1], _f32, kind="ExternalOutput")

    rg = [list(range(N_CORES))]

    with tile.TileContext(nc) as tc:
        with (
            tc.tile_pool(name="const", bufs=1) as cpool,
            tc.tile_pool(name="xg", bufs=6) as xgpool,
            tc.tile_pool(name="work", bufs=6) as wpool,
            tc.tile_pool(name="psum", bufs=1, space="PSUM") as ppool,
            tc.tile_pool(name="dram", bufs=1, space="DRAM") as dpool,
        ):
            # ---- constants / persistent state ----
            iota_sb = cpool.tile([P, CPL], _f16, tag="iota")
            nc.sync.dma_start(iota_sb[:], iota_d.ap())

            ones_f32 = cpool.tile([P, 1], _f32, tag="ones_f32")
            nc.vector.memset(ones_f32[:], 1.0)
            # [p, 2, 16] DoubleRow ones-weight (r stride 16B satisfies the
            # step%16 constraint on weight APs)
            ones_f8 = cpool.tile([P, 32], _f8, tag="ones_f8")
            nc.vector.memset(ones_f8[:], 1.0)

            # sample (g, p, j): row = g*(P*G) + p*G + j; tile k = g*G + j.
            t_i32 = cpool.tile([P, NT], _i32, tag="t_i32")
            nc.sync.dma_start(
                t_i32[:].rearrange("p (g j) -> p g j", j=G),
                t_d.ap().rearrange("(g p j) -> p g j", p=P, j=G))
            t_f32 = cpool.tile([P, NT], _f32, tag="t_f32")
            nc.vector.tensor_copy(t_f32[:], t_i32[:])

            # all supertile one-hots, r-major: [p, u, r, CPL] fp8 (exact 0/1)
            o_all = cpool.tile([P, NU * 2 * CPL], _f8, tag="o_all")

            # running sum-of-squares accumulator [P, 1] f32
            sq_acc = cpool.tile([P, 1], _f32, tag="sq_acc")
            nc.vector.memset(sq_acc[:], 0.0)

            # single PSUM bank accumulates the local segment sums S[128, 512]
            psum_s = ppool.tile([P, D], _f32, tag="s0", name="psum_s")
            # counts bank, fed by in-loop DoubleRow ones-matmuls
            ones3 = ones_f8[:].rearrange("p (r m) -> p r m", r=2)
            cnt_ps = ppool.tile([16, CPL], _f32, tag="s1", name="cnt_ps")

            xga = x_d.ap().rearrange("(g p j) d -> g p j d", p=P, j=G)

            # ---- main loop (DMA-paced; PE is ~3% busy, HAM state moot) ----
            def main_loop():
                for g in range(NG):
                    xg = xgpool.tile([P, G * D], _f32, tag="xg")
                    nc.sync.dma_start(
                        xg[:].rearrange("p (j d) -> p j d", j=G), xga[g])

                    # one ACT Square + accum for the whole 2048-elem group
                    sqs = wpool.tile([P, G * D], _f8e5, tag="sqs")
                    sqp = wpool.tile([P, 1], _f32, tag="sqp")
                    nc.scalar.activation(
                        sqs[:], xg[:], mybir.ActivationFunctionType.Square,
                        accum_out=sqp[:, 0:1])
                    nc.vector.tensor_tensor(
                        sq_acc[:], sq_acc[:], sqp[:], mybir.AluOpType.add)

                    for h in range(G // 2):
                        u = g * (G // 2) + h
                        xv2 = xg[:, h * 2 * D:(h + 1) * 2 * D]   # [P, 1024]

                        # fp8 cast on DVE (ACT is the busier engine here)
                        xf8 = wpool.tile([P, 2 * D], _f8, tag="xf8")
                        nc.vector.tensor_copy(xf8[:], xv2)

                        # fp8 local one-hots, r-major contiguous [p, r, CPL]
                        ob = o_all[:, u * 2 * CPL:(u + 1) * 2 * CPL]
                        for r in range(2):
                            nc.vector.tensor_scalar(
                                ob[:, r * CPL:(r + 1) * CPL], iota_sb[:],
                                t_f32[:, 2 * u + r:2 * u + r + 1], None,
                                mybir.AluOpType.is_equal,
                            )

                        # ONE segment-sum DoubleRow matmul (K=256) per
                        # supertile — classes are core-local.
                        x3 = xf8[:].rearrange("p (r d) -> p r d", r=2)
                        o3 = ob.rearrange("p (r c) -> p r c", r=2)
                        nc.tensor.matmul(
                            psum_s[:], lhsT=o3, rhs=x3,
                            perf_mode=mybir.MatmulPerfMode.DoubleRow,
                            start=(u == 0), stop=(u == NU - 1),
                        )
                        # counts accumulate on the ~97%-idle PE as we go
                        nc.tensor.matmul(
                            cnt_ps[:], lhsT=ones3, rhs=o3,
                            perf_mode=mybir.MatmulPerfMode.DoubleRow,
                            start=(u == 0), stop=(u == NU - 1),
                        )

            if repeat == 1:
                main_loop()
            else:
                with tc.For_i(0, repeat, 1):
                    main_loop()

            # ---- epilogue: S, counts, local term, scalar collective ----
            s_sb = cpool.tile([P, D], _f32, tag="s_sb")
            nc.vector.tensor_copy(s_sb[:], psum_s[:])

            cnt_sb = cpool.tile([1, CPL], _f32, tag="cnt_sb")
            nc.vector.tensor_copy(cnt_sb[:], cnt_ps[0:1, :])
            # transpose counts to [CPL(part), 1] via SBUF->SBUF DMA
            cnt_col = cpool.tile([P, 1], _f32, tag="cnt_col")
            nc.sync.dma_start(cnt_col[:, 0:1], cnt_sb[0:1, :])

            # sumsq partial: reduce [P, 1] across partitions
            sq_ps = ppool.tile([1, 1], _f32, tag="s2", name="sq_ps")
            nc.tensor.matmul(sq_ps[:], lhsT=ones_f32[:], rhs=sq_acc[:, 0:1],
                             start=True, stop=True)

            # local class-shard term: sum_c ||S_c||^2 / max(n_c, 1)
            q = cpool.tile([P, 1], _f32, tag="q")
            qscr = wpool.tile([P, D], _f32, tag="qscr")
            nc.vector.tensor_tensor(qscr[:], s_sb[:], s_sb[:],
                                    mybir.AluOpType.mult)
            nc.vector.tensor_reduce(q[:, 0:1], qscr[:],
                                    axis=mybir.AxisListType.X,
                                    op=mybir.AluOpType.add)
            nmax = cpool.tile([P, 1], _f32, tag="nmax")
            nc.vector.tensor_scalar_max(nmax[:], cnt_col[:], 1.0)
            rinv = cpool.tile([P, 1], _f32, tag="rinv")
            nc.vector.reciprocal(rinv[:], nmax[:])
            bpart = cpool.tile([P, 1], _f32, tag="bpart")
            nc.vector.tensor_tensor(bpart[:], q[:], rinv[:],
                                    mybir.AluOpType.mult)
            b_ps = ppool.tile([1, 1], _f32, tag="s3", name="b_ps")
            nc.tensor.matmul(b_ps[:], lhsT=ones_f32[:], rhs=bpart[:, 0:1],
                             start=True, stop=True)

            # ---- scalar cross-core reduce: RS of replicated [8, 2] ----
            par_sb = cpool.tile([1, 2 * N_CORES], _f32, tag="par_sb")
            for m in range(N_CORES):
                nc.vector.tensor_copy(par_sb[0:1, 2 * m:2 * m + 1], sq_ps[:])
                nc.vector.tensor_copy(par_sb[0:1, 2 * m + 1:2 * m + 2],
                                      b_ps[:])
            rs2_in = dpool.tile([N_CORES, 2], _f32, tag="rs2_in")
            nc.sync.dma_start(rs2_in[:], par_sb[:])
            rs2_out = dpool.tile([1, 2], _f32, tag="rs2_out")
            nc.gpsimd.collective_compute(
                "ReduceScatter", mybir.AluOpType.add, replica_groups=rg,
                ins=[rs2_in.opt()], outs=[rs2_out.opt()],
            )
            fin = cpool.tile([1, 2], _f32, tag="fin")
            nc.sync.dma_start(fin[:], rs2_out[:])

            loss_sb = cpool.tile([1, 1], _f32, tag="loss_sb")
            nc.vector.tensor_tensor(loss_sb[:], fin[0:1, 0:1], fin[0:1, 1:2],
                                    mybir.AluOpType.subtract)
            nc.vector.tensor_scalar_mul(loss_sb[:], loss_sb[:], 0.5)
            nc.sync.dma_start(out_d.ap(), loss_sb[:])

    nc.compile()
    return nc


def _get_compiled():
    global _compiled
    if _compiled is None:
        _compiled = _build()
    return _compiled


_IOTA = np.tile(np.arange(CPL, dtype=np.float16), (P, 1))


def make_in_maps(inputs, targets):
    x = np.asarray(inputs, dtype=np.float32)
    t = np.asarray(targets).astype(np.int32)
    assert x.shape == (B, D) and t.shape == (B,)
    # Route each sample to the core owning its class (classes are
    # contiguously sharded: core m owns [128m, 128(m+1))).
    order = np.argsort(t >> 7, kind="stable")
    counts = np.bincount(t >> 7, minlength=N_CORES)
    assert counts.max() <= BLP, f"class-shard overflow: {counts.max()} > {BLP}"
    offs = np.zeros(N_CORES + 1, np.int64)
    np.cumsum(counts, out=offs[1:])
    in_maps = []
    for c in range(N_CORES):
        rows = order[offs[c]:offs[c + 1]]
        xs = np.zeros((BLP, D), np.float32)
        xs[:len(rows)] = x[rows]
        ts = np.full((BLP,), -1, np.int32)
        ts[:len(rows)] = t[rows] - c * CPL
        in_maps.append({"x": xs, "t": ts, "iota": _IOTA})
    return in_maps


def kernel(inputs, targets, num_classes=C, **_ignored):
    assert int(num_classes) == C
    nc = _get_compiled()
    res = bass_utils.run_bass_kernel_spmd(
        nc, make_in_maps(inputs, targets), core_ids=list(range(N_CORES)))
    return np.asarray(res.results[0]["out"], dtype=np.float32).reshape(())


# revision 3
# speedup vs baseline: 1.0491x; 1.0186x over previous
"""CenterLoss kernel for Trainium2, 8 NeuronCores, sorted-equal-chunk sharded.

Reference computation (B=65536, D=512, C=1024):
    loss = 0.5 * ( sum_i ||x_i||^2  -  sum_c ||S_c||^2 / max(n_c, 1) )
    with S_c = segment_sum(x, t)[c], n_c = bincount(t)[c]   (exact rewrite)

Sharding: rows are sorted by class (host-side, a sharding choice) and cut
into 8 EQUAL chunks of 8192 — zero padding, so the per-core HBM stream is
exactly B*D*4/8 bytes.  A cut can split at most ONE class between adjacent
cores.  Each core computes segment sums over a 256-class window starting
at its first row's class (window-local targets come from the host; max
span ~130 << 256).  Boundary classes are reconciled in the epilogue:

  - each core extracts its window-pos-0 row ("tail" of the left-boundary
    class) and pos-q row ("head" of its right-boundary class, q host-
    supplied as a one-hot selector) with f32 selector matmuls,
  - a placement matmul routes (left -> rs row m-1, right -> rs row m);
    ReduceScatter-add then hands core i the COMBINED (S, n) of boundary
    i, plus globally-summed (sum_sq, T2_nonboundary) in two replicated
    columns,
  - core i adds ||S||^2/max(n,1) of its boundary (core 7: bscale=0) and
    a second tiny ReduceScatter sums those 8 scalars; every core then
    holds the final loss.  Both cores excluded their boundary positions
    from the local T2 via a host keep-mask, so nothing double-counts.

Main loop per 512-sample group (4 tiles, 1 MiB DMA): ACT Square(accum)
for sum||x||^2, DVE fp8 cast + 256-wide one-hot, and 3 DoubleRow fp8
matmuls per 256-sample supertile (2x segment-sum S[256,512], 1x counts)
on the otherwise-idle PE.  The kernel is purely HBM-DMA-bound.
"""

import numpy as np

from concourse import bass, bacc, tile, mybir, bass_utils

B, D, C = 65536, 512, 1024
N_CORES = 8
P = 128
BLP = B // N_CORES         # 8192 rows per core, no padding
W = 256                    # class-window width (max chunk span ~130)
NT = BLP // P              # 64 tiles
G = 4                      # tiles per DMA group (1 MiB)
NG = NT // G               # 16 groups
NU = NT // 2               # 32 supertiles

_f32 = mybir.dt.float32
_f16 = mybir.dt.float16
_i32 = mybir.dt.int32
_f8 = mybir.dt.float8e4
_f8e5 = mybir.dt.float8e5

_compiled = None


def _build(repeat=1):
    nc = bacc.Bacc("TRN2", target_bir_lowering=False, debug=False,
                   num_devices=N_CORES)

    x_d = nc.dram_tensor("x", [BLP, D], _f32, kind="ExternalInput")
    t_d = nc.dram_tensor("t", [BLP], _i32, kind="ExternalInput")
    iota_d = nc.dram_tensor("iota", [P, W], _f16, kind="ExternalInput")
    keep_d = nc.dram_tensor("keep", [P, 2], _f32, kind="ExternalInput")
    sel_d = nc.dram_tensor("sel", [P, 4], _f32, kind="ExternalInput")
    nmask_d = nc.dram_tensor("nmask", [1, W], _f32, kind="ExternalInput")
    place_d = nc.dram_tensor("place", [2, N_CORES], _f32,
                             kind="ExternalInput")
    bscale_d = nc.dram_tensor("bscale", [1, 1], _f32, kind="ExternalInput")
    out_d = nc.dram_tensor("out", [1, 1], _f32, kind="ExternalOutput")

    rg = [list(range(N_CORES))]

    with tile.TileContext(nc) as tc:
        with (
            tc.tile_pool(name="const", bufs=1) as cpool,
            tc.tile_pool(name="xg", bufs=6) as xgpool,
            tc.tile_pool(name="work", bufs=6) as wpool,
            tc.tile_pool(name="psum", bufs=1, space="PSUM") as ppool,
            tc.tile_pool(name="dram", bufs=1, space="DRAM") as dpool,
        ):
            # ---- constants / persistent state ----
            iota_sb = cpool.tile([P, W], _f16, tag="iota")
            nc.sync.dma_start(iota_sb[:], iota_d.ap())
            keep_sb = cpool.tile([P, 2], _f32, tag="keep")
            nc.sync.dma_start(keep_sb[:], keep_d.ap())
            sel_sb = cpool.tile([P, 4], _f32, tag="sel")
            nc.sync.dma_start(sel_sb[:], sel_d.ap())
            nmask_sb = cpool.tile([1, W], _f32, tag="nmask")
            nc.sync.dma_start(nmask_sb[:], nmask_d.ap())
            place_sb = cpool.tile([2, N_CORES], _f32, tag="place")
            nc.sync.dma_start(place_sb[:], place_d.ap())
            bscale_sb = cpool.tile([1, 1], _f32, tag="bscale")
            nc.sync.dma_start(bscale_sb[:], bscale_d.ap())

            ones_f32 = cpool.tile([P, 1], _f32, tag="ones_f32")
            nc.vector.memset(ones_f32[:], 1.0)
            ones1_8 = cpool.tile([1, N_CORES], _f32, tag="ones1_8")
            nc.vector.memset(ones1_8[:], 1.0)
            # [p, 2, 16] DoubleRow ones-weight (r stride 16B)
            ones_f8 = cpool.tile([P, 32], _f8, tag="ones_f8")
            nc.vector.memset(ones_f8[:], 1.0)
            ones3 = ones_f8[:].rearrange("p (r m) -> p r m", r=2)

            # sample (g, p, j): row = g*(P*G) + p*G + j; tile k = g*G + j.
            t_i32 = cpool.tile([P, NT], _i32, tag="t_i32")
            nc.sync.dma_start(
                t_i32[:].rearrange("p (g j) -> p g j", j=G),
                t_d.ap().rearrange("(g p j) -> p g j", p=P, j=G))
            t_f32 = cpool.tile([P, NT], _f32, tag="t_f32")
            nc.vector.tensor_copy(t_f32[:], t_i32[:])

            # one-hots, r-major: [p, u, r, W] fp8 (exact 0/1)
            o_all = cpool.tile([P, NU * 2 * W], _f8, tag="o_all")

            sq_acc = cpool.tile([P, 1], _f32, tag="sq_acc")
            nc.vector.memset(sq_acc[:], 0.0)

            # PSUM banks: 2x segment sums (window halves) + counts
            psum_sA = ppool.tile([P, D], _f32, tag="sA", name="psum_sA")
            psum_sB = ppool.tile([P, D], _f32, tag="sB", name="psum_sB")
            cnt_ps = ppool.tile([16, W], _f32, tag="cnt", name="cnt_ps")

            xga = x_d.ap().rearrange("(g p j) d -> g p j d", p=P, j=G)

            # ---- main loop (DMA-paced; PE ~6% busy) ----
            def main_loop():
                for g in range(NG):
                    xg = xgpool.tile([P, G * D], _f32, tag="xg")
                    nc.sync.dma_start(
                        xg[:].rearrange("p (j d) -> p j d", j=G), xga[g])

                    sqs = wpool.tile([P, G * D], _f8e5, tag="sqs")
                    sqp = wpool.tile([P, 1], _f32, tag="sqp")
                    nc.scalar.activation(
                        sqs[:], xg[:], mybir.ActivationFunctionType.Square,
                        accum_out=sqp[:, 0:1])
                    nc.vector.tensor_tensor(
                        sq_acc[:], sq_acc[:], sqp[:], mybir.AluOpType.add)

                    for h in range(G // 2):
                        u = g * (G // 2) + h
                        xv2 = xg[:, h * 2 * D:(h + 1) * 2 * D]   # [P, 1024]

                        xf8 = wpool.tile([P, 2 * D], _f8, tag="xf8")
                        nc.vector.tensor_copy(xf8[:], xv2)

                        ob = o_all[:, u * 2 * W:(u + 1) * 2 * W]
                        for r in range(2):
                            nc.vector.tensor_scalar(
                                ob[:, r * W:(r + 1) * W], iota_sb[:],
                                t_f32[:, 2 * u + r:2 * u + r + 1], None,
                                mybir.AluOpType.is_equal,
                            )

                        x3 = xf8[:].rearrange("p (r d) -> p r d", r=2)
                        o3 = ob.rearrange("p (r c) -> p r c", r=2)
                        for ci, pb in ((0, psum_sA), (1, psum_sB)):
                            nc.tensor.matmul(
                                pb[:], lhsT=o3[:, :, ci * P:(ci + 1) * P],
                                rhs=x3,
                                perf_mode=mybir.MatmulPerfMode.DoubleRow,
                                start=(u == 0), stop=(u == NU - 1),
                            )
                        nc.tensor.matmul(
                            cnt_ps[:], lhsT=ones3, rhs=o3,
                            perf_mode=mybir.MatmulPerfMode.DoubleRow,
                            start=(u == 0), stop=(u == NU - 1),
                        )

            if repeat == 1:
                main_loop()
            else:
                with tc.For_i(0, repeat, 1):
                    main_loop()

            # ---- epilogue ----
            # flush S and counts; counts also as per-partition columns
            s_sbA = cpool.tile([P, D], _f32, tag="s_sbA")
            nc.vector.tensor_copy(s_sbA[:], psum_sA[:])
            s_sbB = cpool.tile([P, D], _f32, tag="s_sbB")
            nc.vector.tensor_copy(s_sbB[:], psum_sB[:])
            cnt_sb = cpool.tile([1, W], _f32, tag="cnt_sb")
            nc.vector.tensor_copy(cnt_sb[:], cnt_ps[0:1, :])
            cnt_col = cpool.tile([P, 2], _f32, tag="cnt_col")
            nc.sync.dma_start(cnt_col[:, 0:1], cnt_sb[0:1, 0:P])
            nc.sync.dma_start(cnt_col[:, 1:2], cnt_sb[0:1, P:W])

            # boundary-row extraction: ext row 0 = S[pos 0], row 1 = S[pos q]
            ext_ps = ppool.tile([2, D], _f32, tag="ext", name="ext_ps")
            nc.tensor.matmul(ext_ps[:], lhsT=sel_sb[:, 0:2], rhs=s_sbA[:],
                             start=True, stop=False)
            nc.tensor.matmul(ext_ps[:], lhsT=sel_sb[:, 2:4], rhs=s_sbB[:],
                             start=False, stop=True)
            # boundary counts: nL = cnt[0]; nR = sum(cnt * nmask)
            nscr = wpool.tile([1, W], _f32, tag="nscr")
            nc.vector.tensor_tensor(nscr[:], cnt_sb[:], nmask_sb[:],
                                    mybir.AluOpType.mult)
            nR = cpool.tile([1, 1], _f32, tag="nR")
            nc.vector.tensor_reduce(nR[:, 0:1], nscr[:],
                                    axis=mybir.AxisListType.X,
                                    op=mybir.AluOpType.add)
            ext_sb = cpool.tile([2, D + 1], _f32, tag="ext_sb")
            nc.vector.tensor_copy(ext_sb[:, 0:D], ext_ps[:])
            nc.vector.tensor_copy(ext_sb[0:1, D:D + 1], cnt_sb[0:1, 0:1])
            nc.sync.dma_start(ext_sb[1:2, D:D + 1], nR[:])

            # local T2 over kept window positions
            bp_sum = cpool.tile([P, 1], _f32, tag="bp_sum")
            nc.vector.memset(bp_sum[:], 0.0)
            for ti, s_sb in ((0, s_sbA), (1, s_sbB)):
                qscr = wpool.tile([P, D], _f32, tag="qscr")
                nc.vector.tensor_tensor(qscr[:], s_sb[:], s_sb[:],
                                        mybir.AluOpType.mult)
                qv = wpool.tile([P, 1], _f32, tag="qv")
                nc.vector.tensor_reduce(qv[:, 0:1], qscr[:],
                                        axis=mybir.AxisListType.X,
                                        op=mybir.AluOpType.add)
                nmax = wpool.tile([P, 1], _f32, tag="nmax")
                nc.vector.tensor_scalar_max(nmax[:], cnt_col[:, ti:ti + 1],
                                            1.0)
                rinv = wpool.tile([P, 1], _f32, tag="rinv")
                nc.vector.reciprocal(rinv[:], nmax[:])
                bpart = wpool.tile([P, 1], _f32, tag="bpart")
                nc.vector.tensor_tensor(bpart[:], qv[:], rinv[:],
                                        mybir.AluOpType.mult)
                nc.vector.tensor_tensor(bpart[:], bpart[:],
                                        keep_sb[:, ti:ti + 1],
                                        mybir.AluOpType.mult)
                nc.vector.tensor_tensor(bp_sum[:], bp_sum[:], bpart[:],
                                        mybir.AluOpType.add)

            # local scalars (sum_sq, T2_nonboundary) in one ones-matmul
            sc_in = cpool.tile([P, 2], _f32, tag="sc_in")
            nc.vector.tensor_copy(sc_in[:, 0:1], sq_acc[:])
            nc.vector.tensor_copy(sc_in[:, 1:2], bp_sum[:])
            sc_ps = ppool.tile([1, 2], _f32, tag="sc", name="sc_ps")
            nc.tensor.matmul(sc_ps[:], lhsT=ones_f32[:], rhs=sc_in[:],
                             start=True, stop=True)
            sc_sb = cpool.tile([1, 2], _f32, tag="sc_sb")
            nc.vector.tensor_copy(sc_sb[:], sc_ps[:])

            # placement: route (left, right) partials to rs rows
            placeS_ps = ppool.tile([N_CORES, D], _f32, tag="plS",
                                   name="placeS_ps")
            nc.tensor.matmul(placeS_ps[:], lhsT=place_sb[:],
                             rhs=ext_sb[:, 0:D], start=True, stop=True)
            placeN_ps = ppool.tile([N_CORES, 1], _f32, tag="plN",
                                   name="placeN_ps")
            nc.tensor.matmul(placeN_ps[:], lhsT=place_sb[:],
                             rhs=ext_sb[:, D:D + 1], start=True, stop=True)
            # broadcast local scalars to all 8 rows (K=1 ones-matmul)
            bc_ps = ppool.tile([N_CORES, 2], _f32, tag="bc", name="bc_ps")
            nc.tensor.matmul(bc_ps[:], lhsT=ones1_8[:], rhs=sc_sb[:],
                             start=True, stop=True)

            rs_sb = cpool.tile([N_CORES, D + 3], _f32, tag="rs_sb")
            nc.vector.tensor_copy(rs_sb[:, 0:D], placeS_ps[:])
            nc.vector.tensor_copy(rs_sb[:, D:D + 1], placeN_ps[:])
            nc.vector.tensor_copy(rs_sb[:, D + 1:D + 3], bc_ps[:])

            rs1_in = dpool.tile([N_CORES, D + 3], _f32, tag="rs1_in")
            nc.sync.dma_start(rs1_in[:], rs_sb[:])
            rs1_out = dpool.tile([1, D + 3], _f32, tag="rs1_out")
            nc.gpsimd.collective_compute(
                "ReduceScatter", mybir.AluOpType.add, replica_groups=rg,
                ins=[rs1_in.opt()], outs=[rs1_out.opt()],
            )
            fin1 = cpool.tile([1, D + 3], _f32, tag="fin1")
            nc.sync.dma_start(fin1[:], rs1_out[:])

            # this core's boundary term: ||S||^2 / max(n,1) * bscale
            bscr = wpool.tile([1, D], _f32, tag="bscr")
            nc.vector.tensor_tensor(bscr[:], fin1[0:1, 0:D], fin1[0:1, 0:D],
                                    mybir.AluOpType.mult)
            bred = cpool.tile([1, 1], _f32, tag="bred")
            nc.vector.tensor_reduce(bred[:, 0:1], bscr[:],
                                    axis=mybir.AxisListType.X,
                                    op=mybir.AluOpType.add)
            bn = wpool.tile([1, 1], _f32, tag="bn")
            nc.vector.tensor_scalar_max(bn[:], fin1[0:1, D:D + 1], 1.0)
            bni = wpool.tile([1, 1], _f32, tag="bni")
            nc.vector.reciprocal(bni[:], bn[:])
            nc.vector.tensor_tensor(bred[:], bred[:], bni[:],
                                    mybir.AluOpType.mult)
            nc.vector.tensor_tensor(bred[:], bred[:], bscale_sb[:],
                                    mybir.AluOpType.mult)

            # second tiny RS: sum the 8 boundary scalars everywhere
            rs2_sb = cpool.tile([1, N_CORES], _f32, tag="rs2_sb")
            for m in range(N_CORES):
                nc.vector.tensor_copy(rs2_sb[0:1, m:m + 1], bred[:])
            rs2_in = dpool.tile([N_CORES, 1], _f32, tag="rs2_in")
            nc.sync.dma_start(rs2_in[:], rs2_sb[:])
            rs2_out = dpool.tile([1, 1], _f32, tag="rs2_out")
            nc.gpsimd.collective_compute(
                "ReduceScatter", mybir.AluOpType.add, replica_groups=rg,
                ins=[rs2_in.opt()], outs=[rs2_out.opt()],
            )
            fin2 = cpool.tile([1, 1], _f32, tag="fin2")
            nc.sync.dma_start(fin2[:], rs2_out[:])

            # loss = 0.5 * (sum_sq - T2_nonboundary - T2_boundary)
            loss_sb = cpool.tile([1, 1], _f32, tag="loss_sb")
            nc.vector.tensor_tensor(loss_sb[:], fin1[0:1, D + 1:D + 2],
                                    fin1[0:1, D + 2:D + 3],
                                    mybir.AluOpType.subtract)
            nc.vector.tensor_tensor(loss_sb[:], loss_sb[:], fin2[:],
                                    mybir.AluOpType.subtract)
            nc.vector.tensor_scalar_mul(loss_sb[:], loss_sb[:], 0.5)
            nc.sync.dma_start(out_d.ap(), loss_sb[:])

    nc.compile()
    return nc


def _get_compiled():
    global _compiled
    if _compiled is None:
        _compiled = _build()
    return _compiled


_IOTA = np.tile(np.arange(W, dtype=np.float16), (P, 1))


def make_in_maps(inputs, targets):
    x = np.asarray(inputs, dtype=np.float32)
    t = np.asarray(targets).astype(np.int64)
    assert x.shape == (B, D) and t.shape == (B,)
    order = np.argsort(t, kind="stable")
    ts = t[order]
    in_maps = []
    for m in range(N_CORES):
        lo, hi = m * BLP, (m + 1) * BLP
        Wm = int(ts[lo])
        tl = (ts[lo:hi] - Wm).astype(np.int32)
        assert 0 <= tl.min() and tl.max() < W, "class window overflow"
        # right-boundary class = first class of the next chunk
        q = int(ts[hi] - Wm) if m < N_CORES - 1 else W - 1
        assert 0 < q < W
        keep = np.ones((P, 2), np.float32)
        if m > 0:
            keep[0, 0] = 0.0
        if m < N_CORES - 1:
            keep[q % P, q // P] = 0.0
        sel = np.zeros((P, 4), np.float32)
        sel[0, 0] = 1.0                      # left partial = window pos 0
        sel[q % P, 1 + 2 * (q // P)] = 1.0   # right partial = window pos q
        nmask = np.zeros((1, W), np.float32)
        nmask[0, q] = 1.0
        place = np.zeros((2, N_CORES), np.float32)
        place[0, (m - 1) % N_CORES] = 1.0    # left  -> boundary m-1
        place[1, m] = 1.0                    # right -> boundary m
        bscale = np.full((1, 1), 1.0 if m < N_CORES - 1 else 0.0, np.float32)
        in_maps.append({
            "x": np.ascontiguousarray(x[order[lo:hi]]),
            "t": tl,
            "iota": _IOTA,
            "keep": keep,
            "sel": sel,
            "nmask": nmask,
            "place": place,
            "bscale": bscale,
        })
    return in_maps


def kernel(inputs, targets, num_classes=C, **_ignored):
    assert int(num_classes) == C
    nc = _get_compiled()
    res = bass_utils.run_bass_kernel_spmd(
        nc, make_in_maps(inputs, targets), core_ids=list(range(N_CORES)))
    return np.asarray(res.results[0]["out"], dtype=np.float32).reshape(())
